# revision 5
# baseline (speedup 1.0000x reference)
"""Tacotron-style decoder step on 8 Trainium2 NeuronCores.

Strategy:
  Phase A (data-parallel over batch, 32/core): prenet, GRU, LSA attention,
    context vector. Everything kept in transposed [feature, batch] layout so
    per-partition ACT biases and f32r matmuls line up.
  AllGather #1: per-core [ctxT; h'T] (640x32) -> full xcat (5120x32).
  Phase B (model-parallel over LSTM units, 128/core): rnn_in, LSTM1,
    AllGather #2 (h1_new 128x256 -> 1024x256), LSTM2, AllGather #3, then
    mel/stop projections computed redundantly for the full batch.
Host side shards/transposes inputs and reassembles full outputs.
"""
import sys

sys.path.insert(0, "/opt/trn_rl_repo")

import numpy as np
from concourse import bacc, mybir, tile
from concourse import bass_utils

F32 = mybir.dt.float32
F32R = mybir.dt.float32r
AF = mybir.ActivationFunctionType
ALU = mybir.AluOpType
AX = mybir.AxisListType

B, T, DIN, DEC, LSTMD = 256, 400, 512, 128, 1024
NMELS, PRE, NFILT, KS = 80, 256, 32, 31
NCORES = 8
BL = B // NCORES        # 32 local batch
UL = LSTMD // NCORES    # 128 local LSTM units
TP = 416                # padded T for 32-blocks
ENC_BUFS = 10

_CACHE = {}


def _declare(nc):
    I = {}
    O = {}

    def di(name, shape, dt=F32R):
        I[name] = nc.dram_tensor(name, list(shape), dt, kind="ExternalInput").ap()

    def do(name, shape, dt=F32):
        O[name] = nc.dram_tensor(name, list(shape), dt, kind="ExternalOutput").ap()

    # replicated weights
    di("w1T", (NMELS, PRE))
    di("w2T", (PRE, PRE))
    di("gihT", (DIN + PRE, 3 * DEC))
    di("ghhT", (DEC, 3 * DEC))
    di("lsaWT", (DEC, DEC))
    di("LWT", (KS, DEC))
    di("ident", (DEC, DEC))
    di("VMSK", (DEC, BL * BL))
    di("rnnT", (DIN + DEC, LSTMD))
    di("melT", (LSTMD, NMELS))
    di("stopTx", (LSTMD, 1))
    di("stopTc", (DIN, 1))
    di("h1T", (LSTMD, B))
    di("h2T", (LSTMD, B))
    di("vecs", (DEC, 17), F32)
    # per-core
    di("pinT", (NMELS, BL))
    di("hT0", (DEC, BL))
    di("ctxT0", (DIN, BL))
    di("projT", (BL, DEC, T))
    di("enc", (BL, T, DIN))
    di("cumP", (BL, T + KS - 1))
    di("cum", (BL, T), F32)
    di("mask", (BL, T), F32)
    di("l1ihT", (LSTMD, 4 * UL))
    di("l1hhT", (LSTMD, 4 * UL))
    di("l2ihT", (LSTMD, 4 * UL))
    di("l2hhT", (LSTMD, 4 * UL))
    di("lb1", (UL, 4), F32)
    di("lb2", (UL, 4), F32)
    di("c1T", (UL, B), F32)
    di("c2T", (UL, B), F32)
    di("stopb", (1, 1), F32)
    # outputs
    do("scores", (BL, T))
    do("cumn", (BL, T))
    do("attnh", (BL, DEC), F32R)
    do("ctx", (BL, DIN), F32R)
    do("h1n", (UL, B), F32R)
    do("c1n", (UL, B))
    do("h2n", (UL, B), F32R)
    do("c2n", (UL, B))
    do("melsT", (NMELS, B))
    do("stopT", (1, B))
    return I, O


def _program(nc, tc, I, O):
    import dataclasses

    with (
        tc.tile_pool(name="const", bufs=1) as cp,
        tc.tile_pool(name="wp", bufs=1) as wp,
        tc.tile_pool(name="hw", bufs=8) as hwp,
        tc.tile_pool(name="lw", bufs=1) as lwp,
        tc.tile_pool(name="xk", bufs=8) as xkp,
        tc.tile_pool(name="work", bufs=1) as wk,
        tc.tile_pool(name="wk3", bufs=3) as wk3,
        tc.tile_pool(name="encp", bufs=ENC_BUFS) as encp,
        tc.tile_pool(name="tps", bufs=2, space="PSUM") as tps,
        tc.tile_pool(name="atps", bufs=2, space="PSUM") as atps,
        tc.tile_pool(name="acc", bufs=1, space="PSUM") as accp,
        tc.tile_pool(name="gat", bufs=2, space="PSUM") as gatp,
        tc.tile_pool(name="dram", bufs=1, space="DRAM") as dp,
    ):
        # ---------- load small constant/weight tiles ----------
        def load(pool, name, shape, dt=F32R, src=None, tag=None):
            t = pool.tile(list(shape), dt, tag=tag or name)
            nc.sync.dma_start(t[:], src if src is not None else I[name])
            return t

        w1T = load(cp, "w1T", (NMELS, PRE))
        w2T_0 = load(cp, "w2T0", (128, PRE), src=I["w2T"][0:128, :])
        w2T_1 = load(cp, "w2T1", (128, PRE), src=I["w2T"][128:256, :])
        gih = [
            load(cp, f"gih{c}", (128, 3 * DEC), src=I["gihT"][128 * c : 128 * (c + 1), :])
            for c in range(6)
        ]
        ghh = load(cp, "ghhT", (DEC, 3 * DEC))
        lsaWT = load(cp, "lsaWT", (DEC, DEC))
        LWT = load(cp, "LWT", (KS, DEC))
        ident = load(cp, "ident", (DEC, DEC))
        VMSK = load(cp, "VMSK", (DEC, BL * BL))
        vecs = load(cp, "vecs", (DEC, 17), F32)
        pinT = load(cp, "pinT", (NMELS, BL))
        hT0 = load(cp, "hT0", (DEC, BL))
        ctxT0 = [
            load(cp, f"ctxT0{c}", (128, BL), src=I["ctxT0"][128 * c : 128 * (c + 1), :])
            for c in range(4)
        ]
        cum_sb = load(cp, "cum", (BL, T), F32)
        mask_sb = load(cp, "mask", (BL, T), F32)
        lb1 = load(cp, "lb1", (UL, 4), F32)
        lb2 = load(cp, "lb2", (UL, 4), F32)
        c1T = load(cp, "c1T", (UL, B), F32)
        c2T = load(cp, "c2T", (UL, B), F32)
        stopb = load(cp, "stopb", (1, 1), F32)
        rnnT = [
            load(wp, f"rnnT{c}", (128, LSTMD), src=I["rnnT"][128 * c : 128 * (c + 1), :])
            for c in range(5)
        ]
        melT = [
            load(wp, f"melT{c}", (128, NMELS), src=I["melT"][128 * c : 128 * (c + 1), :])
            for c in range(8)
        ]
        stopTx = [
            load(wp, f"stopTx{c}", (128, 1), src=I["stopTx"][128 * c : 128 * (c + 1), :])
            for c in range(8)
        ]
        stopTc = [
            load(wp, f"stopTc{c}", (128, 1), src=I["stopTc"][128 * c : 128 * (c + 1), :])
            for c in range(4)
        ]

        def bias(col):
            return vecs[:, col : col + 1]

        # ---------- prenet ----------
        pout = []
        h1pre = []
        for c in range(2):
            ps = tps.tile([128, BL], F32, tag="tps")
            nc.tensor.matmul(ps[:], w1T[:, 128 * c : 128 * (c + 1)], pinT[:], start=True, stop=True)
            h = wk.tile([128, BL], F32R, tag=f"pre{c}")
            nc.scalar.activation(h[:], ps[:], AF.Relu, bias=bias(c))
            h1pre.append(h)
        for c in range(2):
            ps = tps.tile([128, BL], F32, tag="tps")
            for a in range(2):
                nc.tensor.matmul(
                    ps[:],
                    (w2T_0 if a == 0 else w2T_1)[:, 128 * c : 128 * (c + 1)],
                    h1pre[a][:],
                    start=(a == 0),
                    stop=(a == 1),
                )
            h = wk.tile([128, BL], F32R, tag=f"pout{c}")
            nc.scalar.activation(h[:], ps[:], AF.Relu, bias=bias(2 + c))
            pout.append(h)

        xcat = ctxT0 + pout  # 6 chunks of [128, BL]

        # ---------- GRU ----------
        def gru_gate_psum(g):
            ps = tps.tile([128, BL], F32, tag="tps")
            for c in range(6):
                nc.tensor.matmul(
                    ps[:], gih[c][:, 128 * g : 128 * (g + 1)], xcat[c][:],
                    start=(c == 0), stop=False,
                )
            nc.tensor.matmul(
                ps[:], ghh[:, 128 * g : 128 * (g + 1)], hT0[:], start=False, stop=True
            )
            return ps

        ps_r = gru_gate_psum(0)
        r_sb = wk.tile([128, BL], F32, tag="r_sb")
        nc.scalar.activation(r_sb[:], ps_r[:], AF.Tanh, bias=bias(4), scale=0.5)
        nc.vector.tensor_scalar(r_sb[:], r_sb[:], 0.5, 0.5, ALU.mult, ALU.add)

        ps_z = gru_gate_psum(1)
        z_sb = wk.tile([128, BL], F32, tag="z_sb")
        nc.scalar.activation(z_sb[:], ps_z[:], AF.Tanh, bias=bias(5), scale=0.5)
        nc.vector.tensor_scalar(z_sb[:], z_sb[:], 0.5, 0.5, ALU.mult, ALU.add)

        ps_in = tps.tile([128, BL], F32, tag="tps")
        for c in range(6):
            nc.tensor.matmul(
                ps_in[:], gih[c][:, 256:384], xcat[c][:], start=(c == 0), stop=(c == 5)
            )
        ps_hn = tps.tile([128, BL], F32, tag="tps")
        nc.tensor.matmul(ps_hn[:], ghh[:, 256:384], hT0[:], start=True, stop=True)
        t1 = wk.tile([128, BL], F32, tag="t1")
        nc.scalar.activation(t1[:], ps_hn[:], AF.Identity, bias=bias(6))
        t2 = wk.tile([128, BL], F32, tag="t2")
        nc.vector.tensor_tensor(t2[:], r_sb[:], t1[:], op=ALU.mult)
        t3 = wk.tile([128, BL], F32, tag="t3")
        nc.vector.tensor_tensor(t3[:], ps_in[:], t2[:], op=ALU.add)
        n_sb = wk.tile([128, BL], F32, tag="n_sb")
        nc.scalar.activation(n_sb[:], t3[:], AF.Tanh, bias=bias(7))
        t4 = wk.tile([128, BL], F32, tag="t4")
        nc.vector.tensor_tensor(t4[:], hT0[:].bitcast(F32), n_sb[:], op=ALU.subtract)
        t5 = wk.tile([128, BL], F32, tag="t5")
        nc.vector.tensor_tensor(t5[:], z_sb[:], t4[:], op=ALU.mult)
        hTn = wk.tile([128, BL], F32R, tag="hTn")
        nc.vector.tensor_tensor(hTn[:], n_sb[:], t5[:], op=ALU.add)

        # attn_hidden output (transpose to [BL, DEC])
        attnh = wk.tile([BL, DEC], F32R, tag="attnh")
        for j in range(4):
            nc.vector.transpose(
                attnh[0:32, 32 * j : 32 * (j + 1)].bitcast(F32),
                hTn[32 * j : 32 * (j + 1), 0:32].bitcast(F32),
            )
        nc.sync.dma_start(O["attnh"], attnh[:])

        # ---------- processed query ----------
        ps_q = tps.tile([128, BL], F32, tag="tps")
        nc.tensor.matmul(ps_q[:], lsaWT[:], hTn[:], start=True, stop=True)
        qb = wk.tile([128, BL], F32, tag="qb")
        nc.scalar.activation(qb[:], ps_q[:], AF.Identity, bias=bias(8))

        # ---------- attention per-b ----------
        u_acc = accp.tile([BL, T], F32, tag="acc")
        for b in range(BL):
            pat = wk3.tile([KS, T], F32R, tag="pat")
            win = I["cumP"][b : b + 1, :]
            w = dataclasses.replace(win, ap=[[1, KS], [1, T]])
            nc.sync.dma_start(pat[:], w)
            projb = wk3.tile([DEC, T], F32R, tag="projb")
            nc.sync.dma_start(projb[:], I["projT"][b])
            ps_b = atps.tile([DEC, T], F32, tag="attps")
            nc.tensor.matmul(ps_b[:], LWT[:], pat[:], start=True, stop=False)
            nc.tensor.matmul(ps_b[:], ident[:], projb[:], start=False, stop=True)
            tnh = wk3.tile([DEC, T], F32R, tag="tnh")
            nc.scalar.activation(tnh[:], ps_b[:], AF.Tanh, bias=qb[:, b : b + 1])
            nc.tensor.matmul(
                u_acc[:], VMSK[:, BL * b : BL * (b + 1)], tnh[:],
                start=(b == 0), stop=(b == BL - 1),
            )

        # ---------- softmax over T ----------
        u_sb = wk.tile([BL, T], F32, tag="u_sb")
        nc.vector.tensor_copy(u_sb[:], u_acc[:])
        um = wk.tile([BL, T], F32, tag="um")
        nc.vector.tensor_tensor(um[:], u_sb[:], mask_sb[:], op=ALU.mult)
        nmax = wk.tile([BL, 1], F32, tag="nmax")
        nc.vector.tensor_reduce(nmax[:], um[:], axis=AX.X, op=ALU.max, negate=True)
        ex = wk.tile([BL, T], F32, tag="ex")
        nc.scalar.activation(ex[:], um[:], AF.Exp, bias=nmax[:, 0:1])
        ssum = wk.tile([BL, 1], F32, tag="ssum")
        nc.vector.tensor_reduce(ssum[:], ex[:], axis=AX.X, op=ALU.add)
        rinv = wk.tile([BL, 1], F32, tag="rinv")
        nc.vector.reciprocal(rinv[:], ssum[:])
        sc = wk.tile([BL, TP], F32, tag="sc")
        nc.vector.memset(sc[:, T:TP], 0.0)
        nc.vector.tensor_scalar(sc[:, 0:T], ex[:], rinv[:, 0:1], None, ALU.mult)
        nc.sync.dma_start(O["scores"], sc[:, 0:T])
        cumn = wk.tile([BL, T], F32, tag="cumn")
        nc.vector.tensor_tensor(cumn[:], cum_sb[:], sc[:, 0:T], op=ALU.add)
        nc.sync.dma_start(O["cumn"], cumn[:])

        # ---------- scoresT + masked score tiles ----------
        sct = []
        for c in range(3):
            t = wk.tile([128, 32], F32, tag=f"sct{c}")
            for j in range(4):
                nc.vector.transpose(
                    t[32 * j : 32 * (j + 1), 0:32],
                    sc[0:32, 128 * c + 32 * j : 128 * c + 32 * (j + 1)],
                )
            sct.append(t)
        t = wk.tile([32, 32], F32, tag="sct3")
        nc.vector.transpose(t[0:32, 0:32], sc[0:32, 384:416])
        sct.append(t)

        KCH = [128, 128, 128, 16]
        smsk = []
        for c in range(4):
            kk = 128 if c < 3 else 16
            s = wk.tile([kk, BL * (BL + 1)], F32R, tag=f"smsk{c}")
            nc.vector.memset(s[:].bitcast(F32), 0.0)
            nc.vector.tensor_copy(
                s[:, 0 : BL * (BL + 1) : BL + 1], sct[c][0:kk, 0:BL]
            )
            smsk.append(s)

        # ---------- context ----------
        ctx_acc = accp.tile([BL, DIN], F32, tag="acc")
        first = True
        for b in range(BL):
            for c in range(4):
                kk = KCH[c]
                et = encp.tile([128, DIN], F32R, tag="enc")
                nc.sync.dma_start(et[0:kk, :], I["enc"][b, 128 * c : 128 * c + kk, :])
                nc.tensor.matmul(
                    ctx_acc[:], smsk[c][0:kk, BL * b : BL * (b + 1)], et[0:kk, :],
                    start=first, stop=(b == BL - 1 and c == 3),
                )
                first = False
        ctx_sb = wk.tile([BL, DIN], F32R, tag="ctx_sb")
        nc.vector.tensor_copy(ctx_sb[:], ctx_acc[:])
        nc.sync.dma_start(O["ctx"], ctx_sb[:])
        ctxT = []
        for c in range(4):
            t = wk.tile([128, 32], F32R, tag=f"ctxT{c}")
            for j in range(4):
                nc.vector.transpose(
                    t[32 * j : 32 * (j + 1), 0:32].bitcast(F32),
                    ctx_sb[0:32, 128 * c + 32 * j : 128 * c + 32 * (j + 1)].bitcast(F32),
                )
            ctxT.append(t)

        # ---------- AllGather 1: xcat = [ctxT; hTn] ----------
        cc1i = dp.tile([DIN + DEC, BL], F32R, tag="cc1i")
        cc1o = dp.tile([NCORES * (DIN + DEC), BL], F32R, tag="cc1o")
        for c in range(4):
            nc.sync.dma_start(cc1i[128 * c : 128 * (c + 1), :], ctxT[c][:])
        nc.sync.dma_start(cc1i[DIN : DIN + DEC, :], hTn[:])
        nc.gpsimd.collective_compute(
            "AllGather", ALU.bypass, replica_groups=[list(range(NCORES))],
            ins=[cc1i.opt()], outs=[cc1o.opt()],
        )
        # read back as [640 features x 256 batch] chunks
        xcg = []
        for cc in range(5):
            g = xkp.tile([128, B], F32R, tag="xcg")
            full = cc1o[:, :]
            src = dataclasses.replace(
                full,
                offset=full.offset + 128 * cc * BL,
                ap=[[BL, 128], [(DIN + DEC) * BL, NCORES], [1, BL]],
            )
            nc.sync.dma_start(g[:], src)
            xcg.append(g)

        # ---------- rnn_in: xT chunks ----------
        xT = []
        for m in range(8):
            ps = tps.tile([128, B], F32, tag="tps")
            for c in range(5):
                nc.tensor.matmul(
                    ps[:], rnnT[c][:, 128 * m : 128 * (m + 1)], xcg[c][:],
                    start=(c == 0), stop=(c == 4),
                )
            x = xkp.tile([128, B], F32R, tag="xt")
            nc.scalar.activation(x[:], ps[:], AF.Identity, bias=bias(9 + m))
            xT.append(x)

        # ---------- LSTM helper ----------
        def lstm(ihT_name, hhT_name, hh_rhs, extra_rhs, cT, lb, out_h, out_c):
            """gates from ihT.T@(xT [+ extra]) + hhT.T@hh_rhs; returns h_new tile."""
            ih = [
                load(lwp, f"{ihT_name}{c}", (128, 4 * UL),
                     src=I[ihT_name][128 * c : 128 * (c + 1), :], tag=f"lw_ih{c}")
                for c in range(8)
            ]
            hh = [
                load(lwp, f"{hhT_name}{c}", (128, 4 * UL),
                     src=I[hhT_name][128 * c : 128 * (c + 1), :], tag=f"lw_hh{c}")
                for c in range(8)
            ]
            def gate_psum(g):
                ps = gatp.tile([UL, B], F32, tag="gat")
                n_mm = 16 + (8 if extra_rhs is not None else 0)
                k = 0
                for c in range(8):
                    nc.tensor.matmul(ps[:], ih[c][:, UL * g : UL * (g + 1)], xT[c][:],
                                     start=(k == 0), stop=(k == n_mm - 1)); k += 1
                for c in range(8):
                    nc.tensor.matmul(ps[:], hh[c][:, UL * g : UL * (g + 1)], hh_rhs[c][:],
                                     start=(k == 0), stop=(k == n_mm - 1)); k += 1
                if extra_rhs is not None:
                    for c in range(8):
                        nc.tensor.matmul(ps[:], ih[c][:, UL * g : UL * (g + 1)], extra_rhs[c][:],
                                         start=(k == 0), stop=(k == n_mm - 1)); k += 1
                return ps

            # gate order: i, f, g, o
            def sig(g, col, tag):
                ps = gate_psum(g)
                s = wk.tile([UL, B], F32, tag=tag)
                nc.scalar.activation(s[:], ps[:], AF.Tanh, bias=lb[:, col : col + 1], scale=0.5)
                nc.vector.tensor_scalar(s[:], s[:], 0.5, 0.5, ALU.mult, ALU.add)
                return s

            i_s = sig(0, 0, "i_s")
            f_s = sig(1, 1, "f_s")
            ps_g = gate_psum(2)
            g_s = wk.tile([UL, B], F32, tag="g_s")
            nc.scalar.activation(g_s[:], ps_g[:], AF.Tanh, bias=lb[:, 2:3])
            o_s = sig(3, 3, "o_s")
            tt1 = wk.tile([UL, B], F32, tag="tt1")
            nc.vector.tensor_tensor(tt1[:], f_s[:], cT[:], op=ALU.mult)
            tt2 = wk.tile([UL, B], F32, tag="tt2")
            nc.vector.tensor_tensor(tt2[:], i_s[:], g_s[:], op=ALU.mult)
            cn = wk.tile([UL, B], F32, tag="cn")
            nc.vector.tensor_tensor(cn[:], tt1[:], tt2[:], op=ALU.add)
            nc.sync.dma_start(out_c, cn[:])
            tc_ = wk.tile([UL, B], F32, tag="tc_")
            nc.scalar.activation(tc_[:], cn[:], AF.Tanh)
            hn = wk.tile([UL, B], F32R, tag=ihT_name + "hn")
            nc.vector.tensor_tensor(hn[:], o_s[:], tc_[:], op=ALU.mult)
            nc.sync.dma_start(out_h, hn[:])
            return hn

        h1T_sb = [
            load(hwp, f"h1T{c}", (128, B), src=I["h1T"][128 * c : 128 * (c + 1), :], tag="hT")
            for c in range(8)
        ]
        h1n = lstm("l1ihT", "l1hhT", h1T_sb, None, c1T, lb1, O["h1n"], O["c1n"])

        # AllGather 2: h1n
        cc2i = dp.tile([UL, B], F32R, tag="cc2i")
        cc2o = dp.tile([LSTMD, B], F32R, tag="cc2o")
        nc.sync.dma_start(cc2i[:], h1n[:])
        nc.gpsimd.collective_compute(
            "AllGather", ALU.bypass, replica_groups=[list(range(NCORES))],
            ins=[cc2i.opt()], outs=[cc2o.opt()],
        )
        h1f = []
        for c in range(8):
            g = xkp.tile([128, B], F32R, tag="h1f")
            nc.sync.dma_start(g[:], cc2o[128 * c : 128 * (c + 1), :])
            h1f.append(g)

        h2T_sb = [
            load(hwp, f"h2T{c}", (128, B), src=I["h2T"][128 * c : 128 * (c + 1), :], tag="hT")
            for c in range(8)
        ]
        h2n = lstm("l2ihT", "l2hhT", h2T_sb, h1f, c2T, lb2, O["h2n"], O["c2n"])

        # AllGather 3: h2n
        cc3i = dp.tile([UL, B], F32R, tag="cc3i")
        cc3o = dp.tile([LSTMD, B], F32R, tag="cc3o")
        nc.sync.dma_start(cc3i[:], h2n[:])
        nc.gpsimd.collective_compute(
            "AllGather", ALU.bypass, replica_groups=[list(range(NCORES))],
            ins=[cc3i.opt()], outs=[cc3o.opt()],
        )
        h2f = []
        for c in range(8):
            g = xkp.tile([128, B], F32R, tag="h2f")
            nc.sync.dma_start(g[:], cc3o[128 * c : 128 * (c + 1), :])
            h2f.append(g)

        # ---------- mel + stop over x3 = x + h1f + h2f ----------
        mel_ps = gatp.tile([NMELS, B], F32, tag="gat")
        k = 0
        for src in (xT, h1f, h2f):
            for c in range(8):
                nc.tensor.matmul(mel_ps[:], melT[c][:], src[c][:],
                                 start=(k == 0), stop=(k == 23)); k += 1
        mel_sb = wk.tile([NMELS, B], F32, tag="mel_sb")
        nc.scalar.activation(mel_sb[:], mel_ps[:], AF.Copy)
        nc.sync.dma_start(O["melsT"], mel_sb[:])

        stop_ps = gatp.tile([1, B], F32, tag="gat")
        k = 0
        for src in (xT, h1f, h2f):
            for c in range(8):
                nc.tensor.matmul(stop_ps[:], stopTx[c][:], src[c][:],
                                 start=(k == 0), stop=False); k += 1
        for c in range(4):
            nc.tensor.matmul(stop_ps[:], stopTc[c][:], xcg[c][:],
                             start=False, stop=(c == 3))
        stop_sb = wk.tile([1, B], F32, tag="stop_sb")
        nc.scalar.activation(stop_sb[:], stop_ps[:], AF.Tanh, bias=stopb[0:1, 0:1], scale=0.5)
        nc.vector.tensor_scalar(stop_sb[:], stop_sb[:], 0.5, 0.5, ALU.mult, ALU.add)
        nc.sync.dma_start(O["stopT"], stop_sb[:])


def _build():
    nc = bacc.Bacc("TRN2", target_bir_lowering=False, debug=False, num_devices=NCORES)
    I, O = _declare(nc)
    with tile.TileContext(nc) as tc:
        _program(nc, tc, I, O)
    nc.compile()
    return nc


def _prep(inp):
    f = np.float32

    def T_(x):
        return np.ascontiguousarray(np.asarray(x).T.astype(f))

    enc = np.asarray(inp["encoder_seq"], dtype=f)
    proj = np.asarray(inp["encoder_seq_proj"], dtype=f)
    projT = np.ascontiguousarray(proj.transpose(0, 2, 1))
    cum = np.asarray(inp["cumulative"], dtype=f)
    cumP = np.zeros((B, T + KS - 1), dtype=f)
    cumP[:, (KS - 1) // 2 : (KS - 1) // 2 + T] = cum
    mask = (np.asarray(inp["chars"]) != 0).astype(f)

    LW = (inp["lsa_L"] @ inp["lsa_conv_w"][:, 0, :]).astype(f)       # [128, 31]
    Lb = (inp["lsa_L"] @ inp["lsa_conv_b"]).astype(f)                # [128]
    qbias = (inp["lsa_Wb"] + Lb).astype(f)
    v = np.asarray(inp["lsa_v"][0], dtype=f)
    VMSKa = np.zeros((DEC, BL * BL), dtype=f)
    for b in range(BL):
        VMSKa[:, BL * b + b] = v

    vecs = np.zeros((DEC, 17), dtype=f)
    vecs[:, 0] = inp["prenet_b1"][0:128]
    vecs[:, 1] = inp["prenet_b1"][128:256]
    vecs[:, 2] = inp["prenet_b2"][0:128]
    vecs[:, 3] = inp["prenet_b2"][128:256]
    vecs[:, 4] = 0.5 * (inp["gru_b_ih"][0:128] + inp["gru_b_hh"][0:128])
    vecs[:, 5] = 0.5 * (inp["gru_b_ih"][128:256] + inp["gru_b_hh"][128:256])
    vecs[:, 6] = inp["gru_b_hh"][256:384]
    vecs[:, 7] = inp["gru_b_ih"][256:384]
    vecs[:, 8] = qbias
    rb = np.asarray(inp["rnn_in_b"], dtype=f)
    for m in range(8):
        vecs[:, 9 + m] = rb[128 * m : 128 * (m + 1)]

    mel_used = inp["mel_w"][0::20, :].astype(f)                      # [80, 1024]
    stop_w = np.asarray(inp["stop_w"], dtype=f)

    common = {
        "w1T": T_(inp["prenet_w1"]),
        "w2T": T_(inp["prenet_w2"]),
        "gihT": T_(inp["gru_w_ih"]),
        "ghhT": T_(inp["gru_w_hh"]),
        "lsaWT": T_(inp["lsa_W"]),
        "LWT": T_(LW),
        "ident": np.eye(DEC, dtype=f),
        "VMSK": VMSKa,
        "rnnT": T_(inp["rnn_in_w"]),
        "melT": T_(mel_used),
        "stopTx": np.ascontiguousarray(stop_w[0, 0:LSTMD].astype(f)[:, None]),
        "stopTc": np.ascontiguousarray(stop_w[0, LSTMD:].astype(f)[:, None]),
        "h1T": T_(inp["rnn1_hidden"]),
        "h2T": T_(inp["rnn2_hidden"]),
        "vecs": vecs,
        "stopb": np.asarray(inp["stop_b"], dtype=f).reshape(1, 1),
    }

    c1T_full = T_(inp["rnn1_cell"])
    c2T_full = T_(inp["rnn2_cell"])
    lb_full1 = (np.asarray(inp["lstm1_b_ih"]) + np.asarray(inp["lstm1_b_hh"])).astype(f)
    lb_full2 = (np.asarray(inp["lstm2_b_ih"]) + np.asarray(inp["lstm2_b_hh"])).astype(f)
    w1ih = np.asarray(inp["lstm1_w_ih"], dtype=f)
    w1hh = np.asarray(inp["lstm1_w_hh"], dtype=f)
    w2ih = np.asarray(inp["lstm2_w_ih"], dtype=f)
    w2hh = np.asarray(inp["lstm2_w_hh"], dtype=f)

    in_maps = []
    for k in range(NCORES):
        bs = slice(BL * k, BL * (k + 1))
        us = np.concatenate([np.arange(g * LSTMD + UL * k, g * LSTMD + UL * (k + 1)) for g in range(4)])
        lb1 = np.stack(
            [0.5 * lb_full1[us[0:UL]], 0.5 * lb_full1[us[UL:2*UL]],
             lb_full1[us[2*UL:3*UL]], 0.5 * lb_full1[us[3*UL:4*UL]]], axis=1)
        lb2 = np.stack(
            [0.5 * lb_full2[us[0:UL]], 0.5 * lb_full2[us[UL:2*UL]],
             lb_full2[us[2*UL:3*UL]], 0.5 * lb_full2[us[3*UL:4*UL]]], axis=1)
        m = dict(common)
        m.update({
            "pinT": T_(inp["prenet_in"][bs]),
            "hT0": T_(inp["attn_hidden"][bs]),
            "ctxT0": T_(inp["context_vec"][bs]),
            "projT": np.ascontiguousarray(projT[bs]),
            "enc": np.ascontiguousarray(enc[bs]),
            "cumP": np.ascontiguousarray(cumP[bs]),
            "cum": np.ascontiguousarray(cum[bs]),
            "mask": np.ascontiguousarray(mask[bs]),
            "l1ihT": np.ascontiguousarray(w1ih[us, :].T),
            "l1hhT": np.ascontiguousarray(w1hh[us, :].T),
            "l2ihT": np.ascontiguousarray(w2ih[us, :].T),
            "l2hhT": np.ascontiguousarray(w2hh[us, :].T),
            "lb1": np.ascontiguousarray(lb1),
            "lb2": np.ascontiguousarray(lb2),
            "c1T": np.ascontiguousarray(c1T_full[UL * k : UL * (k + 1), :]),
            "c2T": np.ascontiguousarray(c2T_full[UL * k : UL * (k + 1), :]),
        })
        in_maps.append(m)
    return in_maps


def _assemble(results):
    f = np.float32
    mels = results[0]["melsT"].T.astype(f)[:, :, None]
    scores = np.concatenate([r["scores"] for r in results], axis=0)[:, None, :]
    attnh = np.concatenate([r["attnh"] for r in results], axis=0)
    ctx = np.concatenate([r["ctx"] for r in results], axis=0)
    h1 = np.concatenate([r["h1n"] for r in results], axis=0).T
    h2 = np.concatenate([r["h2n"] for r in results], axis=0).T
    c1 = np.concatenate([r["c1n"] for r in results], axis=0).T
    c2 = np.concatenate([r["c2n"] for r in results], axis=0).T
    stop = results[0]["stopT"].T.astype(f)
    cumn = np.concatenate([r["cumn"] for r in results], axis=0)
    return (
        np.ascontiguousarray(mels), np.ascontiguousarray(scores),
        np.ascontiguousarray(attnh), np.ascontiguousarray(h1),
        np.ascontiguousarray(h2), np.ascontiguousarray(c1),
        np.ascontiguousarray(c2), np.ascontiguousarray(ctx),
        np.ascontiguousarray(stop), np.ascontiguousarray(cumn),
    )


def kernel(**inputs):
    nc = _CACHE.get("nc")
    if nc is None:
        nc = _build()
        _CACHE["nc"] = nc
    in_maps = _prep(inputs)
    res = bass_utils.run_bass_kernel_spmd(nc, in_maps, core_ids=list(range(NCORES)))
    return _assemble(res.results)
